# revision 1
# baseline (speedup 1.0000x reference)
"""Trainium2 Bass kernel for nn_MLoss_68066641707785 (topk_masking loss).

Computes, for x, y of shape [128, 43264, 5] (fp32):
    m        = (y[:,:,0] > 0.5)
    face_num = sum(m)
    scale    = 1 + 1/face_num
    diff_box = scale * sum(m * (x[:,:,1:5]-y[:,:,1:5])^2) / (face_num*4)
    bce      = -(t*log(p) + (1-t)*log(1-p)),  p = x[:,:,0], t = y[:,:,0]
    diff_c   = scale * sum(m * bce) / face_num
    diff_bg  = 0.5 * mean(-log(1-p))
    out      = diff_box + diff_c + diff_bg          (scalar fp32)

Strategy: pure data-parallel over the batch axis (16 batches per core x 8
cores).  The host first de-interleaves each tensor into a contiguous
confidence plane [B,N] and box plane [B,N,4] so every on-device access is
unit-stride (a stride-5 access pattern runs at ~0.5 elem/cycle on DVE and
~0.25 on ACT).  Each core streams its ~27.7MB through SBUF in T tiles and
reduces on-chip to six [128, T] partial-sum strips:
    aS : sum(m*t)            bS : sum(m*(1-t))      (aS+bS = face count)
    s1 : sum(m*t*ln(p))      s2 : sum(m*(1-t)*ln(1-p))
    se : sum(m * sum_c (x_c-y_c)^2)                 (box SE, masked)
    bg : sum(ln(1-p))                               (all cells)
Work is split across engines: ACT does ln/ln/square, DVE does the fused
compare-multiply-accumulate ops (scalar_tensor_tensor) and the channel
reduce, GpSimd takes the box subtract for some tiles to keep DVE below the
~85us DMA floor.  The host sums the 8 cores' strips in float64 and applies
the final scalar formula.
"""

import numpy as np

try:
    from concourse import bacc, bass, mybir, tile
    from concourse.bass_utils import run_bass_kernel_spmd
except ImportError:  # repo not on sys.path in a fresh grading dir
    import sys

    for _p in ("/opt/trn_rl_repo", "/root/.axon_site/_ro/trn_rl_repo"):
        if _p not in sys.path:
            sys.path.insert(0, _p)
    from concourse import bacc, bass, mybir, tile
    from concourse.bass_utils import run_bass_kernel_spmd

THRESH = 0.5
ALPHA = 0.5

B, N, C = 128, 43264, 5
M = 8                      # cores
BS = B // M                # 16 batches per core
P = 128                    # SBUF partitions
CELLS = BS * N // P        # 5408 cells per partition per core
T = 8                      # tiles per core
FT = CELLS // T            # 676 cells per partition per tile
NSTRIP = 5
GP_SUB_TILES = 8           # tiles whose box-subtract runs on GpSimd

_CACHE = {}


def _build():
    f32 = mybir.dt.float32
    AF = mybir.ActivationFunctionType
    OP = mybir.AluOpType
    AX = mybir.AxisListType

    nc = bacc.Bacc("TRN2", target_bir_lowering=False, debug=False, num_devices=M)
    xc_d = nc.declare_dram_parameter("xc", [P, CELLS], f32, isOutput=False)
    yc_d = nc.declare_dram_parameter("yc", [P, CELLS], f32, isOutput=False)
    xb_d = nc.declare_dram_parameter("xb", [P, 4 * CELLS], f32, isOutput=False)
    yb_d = nc.declare_dram_parameter("yb", [P, 4 * CELLS], f32, isOutput=False)
    o_d = nc.declare_dram_parameter("o", [NSTRIP, P, T], f32, isOutput=True)
    xc_ap, yc_ap, xb_ap, yb_ap, o_ap = xc_d[:], yc_d[:], xb_d[:], yb_d[:], o_d[:]

    with tile.TileContext(nc) as tc:
        with tc.tile_pool(name="io", bufs=3) as io, \
             tc.tile_pool(name="mid", bufs=2) as mid, \
             tc.tile_pool(name="acc", bufs=1) as accp:
            faceS = accp.tile([P, T], f32)
            s1S = accp.tile([P, T], f32)
            s2S = accp.tile([P, T], f32)
            seS = accp.tile([P, T], f32)
            bgS = accp.tile([P, T], f32)

            for j in range(T):
                p_t = io.tile([P, FT], f32, tag="p")
                nc.sync.dma_start(out=p_t[:], in_=xc_ap[:, bass.ts(j, FT)])
                t_t = io.tile([P, FT], f32, tag="t")
                nc.sync.dma_start(out=t_t[:], in_=yc_ap[:, bass.ts(j, FT)])
                xb_t = io.tile([P, 4 * FT], f32, tag="xb")
                nc.sync.dma_start(out=xb_t[:], in_=xb_ap[:, bass.ts(j, 4 * FT)])
                yb_t = io.tile([P, 4 * FT], f32, tag="yb")
                nc.sync.dma_start(out=yb_t[:], in_=yb_ap[:, bass.ts(j, 4 * FT)])

                # ---- confidence channel (all unit-stride) ----
                lp = mid.tile([P, FT], f32, tag="lp")
                nc.scalar.activation(lp[:], p_t[:], AF.Ln)
                lq = mid.tile([P, FT], f32, tag="lq")
                nc.scalar.activation(lq[:], p_t[:], AF.Ln, bias=1.0, scale=-1.0,
                                     accum_out=bgS[:, j:j + 1])
                m = mid.tile([P, FT], f32, tag="m")
                nc.vector.tensor_scalar(m[:], t_t[:], THRESH, 0.0, OP.is_gt,
                                        OP.add, accum_out=faceS[:, j:j + 1])
                a = mid.tile([P, FT], f32, tag="a")
                nc.vector.tensor_mul(a[:], m[:], t_t[:])
                b = mid.tile([P, FT], f32, tag="b")
                nc.vector.tensor_sub(b[:], m[:], a[:])
                scr1 = mid.tile([P, FT], f32, tag="scr")
                nc.vector.scalar_tensor_tensor(
                    scr1[:], a[:], 1.0, lp[:], OP.mult, OP.mult,
                    accum_out=s1S[:, j:j + 1])
                scr2 = mid.tile([P, FT], f32, tag="scr")
                nc.vector.scalar_tensor_tensor(
                    scr2[:], b[:], 1.0, lq[:], OP.mult, OP.mult,
                    accum_out=s2S[:, j:j + 1])

                # ---- box channels ----
                d = mid.tile([P, 4 * FT], f32, tag="d", bufs=3)
                sub_eng = nc.gpsimd if j % 4 != 3 else nc.vector
                sub_eng.tensor_sub(d[:], xb_t[:], yb_t[:])
                sq = mid.tile([P, 4 * FT], f32, tag="sq", bufs=3)
                nc.scalar.activation(sq[:], d[:], AF.Square)
                sec = mid.tile([P, FT], f32, tag="sec")
                nc.vector.tensor_reduce(
                    sec[:], sq[:].rearrange("p (f c) -> p f c", c=4),
                    axis=AX.X, op=OP.add)
                scr3 = mid.tile([P, FT], f32, tag="scr")
                nc.vector.scalar_tensor_tensor(
                    scr3[:], m[:], 1.0, sec[:], OP.mult, OP.mult,
                    accum_out=seS[:, j:j + 1])

            for k, strip in enumerate((faceS, s1S, s2S, seS, bgS)):
                nc.sync.dma_start(out=o_ap[k], in_=strip[:])

    nc.compile()
    return nc


def _get_nc():
    if "nc" not in _CACHE:
        _CACHE["nc"] = _build()
    return _CACHE["nc"]


def _in_maps(x, y):
    x = np.asarray(x, dtype=np.float32)
    y = np.asarray(y, dtype=np.float32)
    xc = np.ascontiguousarray(x[:, :, 0])
    yc = np.ascontiguousarray(y[:, :, 0])
    xb = np.ascontiguousarray(x[:, :, 1:5])
    yb = np.ascontiguousarray(y[:, :, 1:5])
    maps = []
    for i in range(M):
        sl = slice(i * BS, (i + 1) * BS)
        maps.append({
            "xc": xc[sl].reshape(P, CELLS),
            "yc": yc[sl].reshape(P, CELLS),
            "xb": xb[sl].reshape(P, 4 * CELLS),
            "yb": yb[sl].reshape(P, 4 * CELLS),
        })
    return maps


def _combine(outs):
    """outs: list of M arrays [NSTRIP, P, T] -> scalar fp32 loss."""
    tot = np.zeros(NSTRIP, dtype=np.float64)
    for o in outs:
        tot += o.astype(np.float64).reshape(NSTRIP, -1).sum(axis=1)
    face, s1, s2, se, bg = tot
    scale = 1.0 + 1.0 / face
    diff_box = scale * se / (face * 4.0)
    diff_c = scale * (-(s1 + s2)) / face
    diff_bg = ALPHA * (-bg) / (B * N)
    return np.asarray(diff_box + diff_c + diff_bg, dtype=np.float32)


def kernel(x, y, **run_kwargs):
    nc = _get_nc()
    res = run_bass_kernel_spmd(nc, _in_maps(x, y), core_ids=list(range(M)),
                               **run_kwargs)
    out = _combine([res.results[i]["o"] for i in range(M)])
    if run_kwargs:
        return out, res
    return out



# revision 7
# speedup vs baseline: 1.3488x; 1.3488x over previous
"""Trainium2 Bass kernel for nn_MLoss_68066641707785 (topk_masking loss).

Computes, for x, y of shape [128, 43264, 5] (fp32):
    m        = (y[:,:,0] > 0.5)
    face_num = sum(m)
    scale    = 1 + 1/face_num
    diff_box = scale * sum(m * (x[:,:,1:5]-y[:,:,1:5])^2) / (face_num*4)
    bce      = -(t*log(p) + (1-t)*log(1-p)),  p = x[:,:,0], t = y[:,:,0]
    diff_c   = scale * sum(m * bce) / face_num
    diff_bg  = 0.5 * mean(-log(1-p))
    out      = diff_box + diff_c + diff_bg          (scalar fp32)

Strategy (v2): pure data-parallel over batch (16 batches/core x 8 cores).
The kernel is HBM-bound, so the host casts everything to bf16 (the 2e-2
rel-err budget dwarfs bf16's ~3e-5 end-to-end impact), halving HBM traffic
to 13.84 MB/core (~38.7us at the 358 GB/s per-core HBM limit).  Host packs
per core two DRAM tensors in channel-planar, chunk-major layout:
    pk  [P, 6*CELLS]: T=4 chunks of [xc | yc | xb0..xb3]   (conf + box-x)
    ybn [P, 4*CELLS]: T=4 chunks of [-yb0..-yb3]           (negated box-y)
Per chunk, three DMA streams run concurrently: conf on the sync HWDGE ring,
box-x on the scalar HWDGE ring, and ybn via gpsimd SWDGE with accum_op=add,
so d = xb - yb materializes in SBUF with zero engine compute.

On-chip per chunk (FC = 1352 cells/partition), all bf16 with fp32 accums:
    ACT : lp = ln(p); lq = ln(1-p) (accum -> bg); sq = Square(e) (accum -> se)
    DVE : mask = (t > 0.5) (accum -> face); dl = lp-lq; u = t*dl; v = u+lq;
          mask*v (accum -> s12, the masked-BCE sum since bce = -(t*dl+lq))
    e_c = d_c * mask (c = 0..3): DVE on chunks 0,3; GpSimd on chunks 1,2
          (keeps each engine under the DMA floor).
The host sums the 8 cores' [P, T] strips in float64 and applies the final
scalar formula.
"""

import numpy as np

try:
    from concourse import bacc, bass, mybir, tile
    from concourse.bass_utils import run_bass_kernel_spmd
except ImportError:  # repo not on sys.path in a fresh grading dir
    import sys

    for _p in ("/opt/trn_rl_repo", "/root/.axon_site/_ro/trn_rl_repo"):
        if _p not in sys.path:
            sys.path.insert(0, _p)
    from concourse import bacc, bass, mybir, tile
    from concourse.bass_utils import run_bass_kernel_spmd

import ml_dtypes

BF16 = ml_dtypes.bfloat16

THRESH = 0.5
ALPHA = 0.5

B, N, C = 128, 43264, 5
M = 8                      # cores
BS = B // M                # 16 batches per core
P = 128                    # SBUF partitions
CELLS = BS * N // P        # 5408 cells per partition per core
T = 4                      # chunks per core
FC = CELLS // T            # 1352 cells per partition per chunk
NSTRIP = 4
GP_ADD_CHUNKS = (1, 2)     # chunks whose box adds run on GpSimd
GP_MUL_CHUNKS = ()         # chunks whose mask-muls run on GpSimd

_CACHE = {}


def _build():
    f32 = mybir.dt.float32
    bf = mybir.dt.bfloat16
    AF = mybir.ActivationFunctionType
    OP = mybir.AluOpType

    nc = bacc.Bacc("TRN2", target_bir_lowering=False, debug=False, num_devices=M)
    pk_d = nc.declare_dram_parameter("pk", [P, 6 * CELLS], bf, isOutput=False)
    ybn_d = nc.declare_dram_parameter("ybn", [P, 4 * CELLS], bf, isOutput=False)
    o_d = nc.declare_dram_parameter("o", [NSTRIP, P, T], f32, isOutput=True)
    pk_ap, ybn_ap, o_ap = pk_d[:], ybn_d[:], o_d[:]

    with tile.TileContext(nc) as tc:
        with tc.tile_pool(name="io", bufs=3) as io, \
             tc.tile_pool(name="mid", bufs=2) as mid, \
             tc.tile_pool(name="acc", bufs=1) as accp:
            faceS = accp.tile([P, T], f32)
            s12S = accp.tile([P, T], f32)
            seS = accp.tile([P, T], f32)
            bgS = accp.tile([P, T], f32)

            for j in range(T):
                base = j * 6 * FC
                ct = io.tile([P, 2 * FC], bf, tag="ct")
                nc.sync.dma_start(out=ct[:], in_=pk_ap[:, base:base + 2 * FC])
                bt = io.tile([P, 4 * FC], bf, tag="bt")
                nc.scalar.dma_start(
                    out=bt[:], in_=pk_ap[:, base + 2 * FC:base + 6 * FC])
                yt = io.tile([P, 4 * FC], bf, tag="yt")
                nc.sync.dma_start(
                    out=yt[:], in_=ybn_ap[:, j * 4 * FC:(j + 1) * 4 * FC])
                add_eng = nc.gpsimd if j in GP_ADD_CHUNKS else nc.vector
                add_eng.tensor_add(bt[:], bt[:], yt[:])

                p_ap = ct[:, 0:FC]
                t_ap = ct[:, FC:2 * FC]

                # ---- confidence channel ----
                lp = mid.tile([P, FC], bf, tag="lp")
                nc.scalar.activation(lp[:], p_ap, AF.Ln)
                lq = mid.tile([P, FC], bf, tag="lq")
                nc.scalar.activation(lq[:], p_ap, AF.Ln, bias=1.0, scale=-1.0,
                                     accum_out=bgS[:, j:j + 1])
                m = mid.tile([P, FC], bf, tag="m")
                nc.vector.tensor_scalar(m[:], t_ap, THRESH, 0.0, OP.is_gt,
                                        OP.add, accum_out=faceS[:, j:j + 1])
                dl = mid.tile([P, FC], bf, tag="dl")
                nc.vector.tensor_sub(dl[:], lp[:], lq[:])
                u = mid.tile([P, FC], bf, tag="u")
                nc.vector.tensor_mul(u[:], t_ap, dl[:])
                v = mid.tile([P, FC], bf, tag="v")
                nc.vector.tensor_add(v[:], u[:], lq[:])
                scr = mid.tile([P, FC], bf, tag="scr")
                nc.vector.scalar_tensor_tensor(
                    scr[:], m[:], 1.0, v[:], OP.mult, OP.mult,
                    accum_out=s12S[:, j:j + 1])

                # ---- box channels: bt already holds d = xb - yb ----
                e = mid.tile([P, 4 * FC], bf, tag="e")
                eng = nc.gpsimd if j in GP_MUL_CHUNKS else nc.vector
                for c in range(4):
                    eng.tensor_mul(e[:, c * FC:(c + 1) * FC],
                                   bt[:, c * FC:(c + 1) * FC], m[:])
                sq = mid.tile([P, 4 * FC], bf, tag="sq")
                nc.scalar.activation(sq[:], e[:], AF.Square,
                                     accum_out=seS[:, j:j + 1])

            for k, strip in enumerate((faceS, s12S, seS, bgS)):
                nc.sync.dma_start(out=o_ap[k], in_=strip[:])

    nc.compile()
    return nc


def _get_nc():
    if "nc" not in _CACHE:
        _CACHE["nc"] = _build()
    return _CACHE["nc"]


def _in_maps(x, y):
    x = np.asarray(x, dtype=np.float32)
    y = np.asarray(y, dtype=np.float32)
    xcf = x[:, :, 0]
    ycf = y[:, :, 0]
    xbf = x[:, :, 1:5]
    ybf = y[:, :, 1:5]
    maps = []
    for i in range(M):
        sl = slice(i * BS, (i + 1) * BS)
        pk = np.empty((P, T, 6, FC), dtype=BF16)
        pk[:, :, 0, :] = xcf[sl].reshape(P, T, FC)
        pk[:, :, 1, :] = ycf[sl].reshape(P, T, FC)
        pk[:, :, 2:6, :] = xbf[sl].reshape(P, T, FC, 4).transpose(0, 1, 3, 2)
        ybn = np.ascontiguousarray(
            (-ybf[sl]).reshape(P, T, FC, 4).transpose(0, 1, 3, 2)).astype(BF16)
        maps.append({
            "pk": pk.reshape(P, 6 * CELLS),
            "ybn": ybn.reshape(P, 4 * CELLS),
        })
    return maps


def _combine(outs):
    """outs: list of M arrays [NSTRIP, P, T] -> scalar fp32 loss."""
    tot = np.zeros(NSTRIP, dtype=np.float64)
    for o in outs:
        tot += o.astype(np.float64).reshape(NSTRIP, -1).sum(axis=1)
    face, s12, se, bg = tot
    scale = 1.0 + 1.0 / face
    diff_box = scale * se / (face * 4.0)
    diff_c = scale * (-s12) / face
    diff_bg = ALPHA * (-bg) / (B * N)
    return np.asarray(diff_box + diff_c + diff_bg, dtype=np.float32)


def kernel(x, y, **run_kwargs):
    nc = _get_nc()
    res = run_bass_kernel_spmd(nc, _in_maps(x, y), core_ids=list(range(M)),
                               **run_kwargs)
    out = _combine([res.results[i]["o"] for i in range(M)])
    if run_kwargs:
        return out, res
    return out


# revision 8
# speedup vs baseline: 1.6494x; 1.2229x over previous
"""Trainium2 Bass kernel for nn_MLoss_68066641707785 (topk_masking loss).

Computes, for x, y of shape [128, 43264, 5] (fp32):
    m        = (y[:,:,0] > 0.5)
    face_num = sum(m)
    scale    = 1 + 1/face_num
    diff_box = scale * sum(m * (x[:,:,1:5]-y[:,:,1:5])^2) / (face_num*4)
    bce      = -(t*log(p) + (1-t)*log(1-p)),  p = x[:,:,0], t = y[:,:,0]
    diff_c   = scale * sum(m * bce) / face_num
    diff_bg  = 0.5 * mean(-log(1-p))
    out      = diff_box + diff_c + diff_bg          (scalar fp32)

Strategy (v4): pure data-parallel over batch (16 batches/core x 8 cores).
The kernel is HBM-bound in fp32, so the host casts to bf16 (2e-2 rel-err
budget vs ~5e-5 bf16 impact), halving HBM traffic to 13.84 MB/core
(~38.7us at the 358 GB/s/core HBM limit).  bf16 also doubles DVE
tensor_tensor throughput (2x_1P mode, measured 855ns @ FD=1352).

Measured constraints that shaped this design:
  - GpSimd compute steals SBUF bandwidth from DVE (concurrent DVE ops run
    ~3x slower), so GpSimd does nothing here.
  - The std tensor_tensor_reduce instr and SWDGE accum-DMAs crash the
    runtime; STT/TS-with-accum run at 1x only.  So accumulating ops are
    minimized and plain 2x TT / 4x TS do the bulk work.
  - ACT (scalar engine) runs 1 elem/cycle @1.2GHz: Ln/Square + the three
    cheap accumulators (bg, se, face) live there, ~34us < DVE ~41us.

Layout (host-packed, per core): conf planes grouped in 2 double-chunks
(xc|yc planar), box planes channel-planar in 4 uneven chunks
(1536,1536,1536,800 cells/partition; small last chunk = short tail).
Per conf group g (FD = 2*FCg):
    ACT: lp = Ln(p); lq = Ln(1-p) (accum -> bg); Identity(m) (accum -> face)
    DVE: m = (t > 0.5) [tensor_scalar, 4x]; dl = lp-lq; u = t*dl; v = u+lq
         [in-place, 2x]; w = m*v [STT, accum -> s12; bce = -(t*dl+lq)]
Per box chunk j (FD = 4*FCj):
    DVE: d = bx - yb [2x, in-place]; e = d * m_broadcast [2x, stride-0 AP]
    ACT: sq = Square(e) (accum -> se_j)
The host sums the strips in float64 and applies the final scalar formula.
"""

import numpy as np

try:
    from concourse import bacc, bass, mybir, tile
    from concourse.bass_utils import run_bass_kernel_spmd
except ImportError:  # repo not on sys.path in a fresh grading dir
    import sys

    for _p in ("/opt/trn_rl_repo", "/root/.axon_site/_ro/trn_rl_repo"):
        if _p not in sys.path:
            sys.path.insert(0, _p)
    from concourse import bacc, bass, mybir, tile
    from concourse.bass_utils import run_bass_kernel_spmd

import ml_dtypes

BF16 = ml_dtypes.bfloat16

THRESH = 0.5
ALPHA = 0.5

B, N, C = 128, 43264, 5
M = 8                      # cores
BS = B // M                # 16 batches per core
P = 128                    # SBUF partitions
CELLS = BS * N // P        # 5408 cells per partition per core
FCS = (1536, 1536, 1536, 800)          # box chunk cells/partition
GROUPS = ((0, 1), (2, 3))              # conf groups = pairs of box chunks
NBOX = len(FCS)
NGRP = len(GROUPS)
# acc strip columns: face[0:2] s12[2:4] se[4:8] bg[8:10]
ACCW = 2 * NGRP + NBOX + 2

_CACHE = {}


def _chunk_off(j):
    return sum(FCS[:j])


def _build():
    f32 = mybir.dt.float32
    bf = mybir.dt.bfloat16
    AF = mybir.ActivationFunctionType
    OP = mybir.AluOpType

    nc = bacc.Bacc("TRN2", target_bir_lowering=False, debug=False, num_devices=M)
    cf_d = nc.declare_dram_parameter("cf", [P, 2 * CELLS], bf, isOutput=False)
    bx_d = nc.declare_dram_parameter("bx", [P, 4 * CELLS], bf, isOutput=False)
    yb_d = nc.declare_dram_parameter("yb", [P, 4 * CELLS], bf, isOutput=False)
    o_d = nc.declare_dram_parameter("o", [P, ACCW], f32, isOutput=True)
    cf_ap, bx_ap, yb_ap, o_ap = cf_d[:], bx_d[:], yb_d[:], o_d[:]

    with tile.TileContext(nc) as tc:
        with tc.tile_pool(name="io", bufs=3) as io, \
             tc.tile_pool(name="mid", bufs=2) as mid, \
             tc.tile_pool(name="acc", bufs=1) as accp:
            acc = accp.tile([P, ACCW], f32)

            m_tiles = {}
            for g, chunks in enumerate(GROUPS):
                fg = sum(FCS[j] for j in chunks)     # cells in this group
                cbase = 2 * _chunk_off(chunks[0])
                ct = io.tile([P, 2 * fg], bf, tag="ct")
                nc.sync.dma_start(out=ct[:], in_=cf_ap[:, cbase:cbase + 2 * fg])
                p_ap = ct[:, 0:fg]
                t_ap = ct[:, fg:2 * fg]

                lp = mid.tile([P, fg], bf, tag="lp")
                nc.scalar.activation(lp[:], p_ap, AF.Ln)
                lq = mid.tile([P, fg], bf, tag="lq")
                nc.scalar.activation(lq[:], p_ap, AF.Ln, bias=1.0, scale=-1.0,
                                     accum_out=acc[:, 8 + g:9 + g])
                m = mid.tile([P, fg], bf, tag="m")
                nc.vector.tensor_scalar(m[:], t_ap, THRESH, None, OP.is_gt)
                scrf = mid.tile([P, fg], bf, tag="scrf")
                nc.scalar.activation(scrf[:], m[:], AF.Identity,
                                     accum_out=acc[:, g:g + 1])
                u = mid.tile([P, fg], bf, tag="u")
                nc.vector.tensor_sub(lp[:], lp[:], lq[:])       # lp <- dl
                nc.vector.tensor_mul(u[:], t_ap, lp[:])
                nc.vector.tensor_add(u[:], u[:], lq[:])         # u <- v
                nc.vector.scalar_tensor_tensor(
                    lp[:], m[:], 1.0, u[:], OP.mult, OP.mult,
                    accum_out=acc[:, 2 + g:3 + g])
                m_tiles[g] = m

                # ---- box chunks of this group ----
                for j in chunks:
                    fc = FCS[j]
                    boff = 4 * _chunk_off(j)
                    bx = io.tile([P, 4 * fc], bf, tag="bx")
                    nc.scalar.dma_start(
                        out=bx[:], in_=bx_ap[:, boff:boff + 4 * fc])
                    yb = io.tile([P, 4 * fc], bf, tag="yb")
                    nc.sync.dma_start(
                        out=yb[:], in_=yb_ap[:, boff:boff + 4 * fc])
                    nc.vector.tensor_sub(bx[:], bx[:], yb[:])   # bx <- d
                    moff = _chunk_off(j) - _chunk_off(chunks[0])
                    m_b = m[:, moff:moff + fc].unsqueeze(1).broadcast_to(
                        (P, 4, fc))
                    nc.vector.tensor_mul(
                        bx[:].rearrange("p (c f) -> p c f", c=4),
                        bx[:].rearrange("p (c f) -> p c f", c=4), m_b)
                    nc.scalar.activation(yb[:], bx[:], AF.Square,
                                         accum_out=acc[:, 4 + j:5 + j])

            nc.sync.dma_start(out=o_ap[:], in_=acc[:])

    nc.compile()
    return nc


def _get_nc():
    if "nc" not in _CACHE:
        _CACHE["nc"] = _build()
    return _CACHE["nc"]


def _in_maps(x, y):
    x = np.asarray(x, dtype=np.float32)
    y = np.asarray(y, dtype=np.float32)
    xcf = x[:, :, 0]
    ycf = y[:, :, 0]
    xbf = x[:, :, 1:5]
    ybf = y[:, :, 1:5]
    maps = []
    for i in range(M):
        sl = slice(i * BS, (i + 1) * BS)
        xc = xcf[sl].reshape(P, CELLS)
        yc = ycf[sl].reshape(P, CELLS)
        cf = np.empty((P, 2 * CELLS), dtype=BF16)
        col = 0
        for chunks in GROUPS:
            f0, f1 = _chunk_off(chunks[0]), _chunk_off(chunks[-1]) + FCS[chunks[-1]]
            fg = f1 - f0
            cf[:, col:col + fg] = xc[:, f0:f1]
            cf[:, col + fg:col + 2 * fg] = yc[:, f0:f1]
            col += 2 * fg
        # box: channel-planar within each chunk
        xb4 = xbf[sl].reshape(P, CELLS, 4)
        yb4 = ybf[sl].reshape(P, CELLS, 4)
        bx = np.empty((P, 4 * CELLS), dtype=BF16)
        yb = np.empty((P, 4 * CELLS), dtype=BF16)
        for j, fc in enumerate(FCS):
            f0 = _chunk_off(j)
            bx[:, 4 * f0:4 * (f0 + fc)] = \
                xb4[:, f0:f0 + fc, :].transpose(0, 2, 1).reshape(P, 4 * fc)
            yb[:, 4 * f0:4 * (f0 + fc)] = \
                yb4[:, f0:f0 + fc, :].transpose(0, 2, 1).reshape(P, 4 * fc)
        maps.append({"cf": cf, "bx": bx, "yb": yb})
    return maps


def _combine(outs):
    """outs: list of M arrays [P, ACCW] -> scalar fp32 loss."""
    tot = np.zeros(ACCW, dtype=np.float64)
    for o in outs:
        tot += o.astype(np.float64).sum(axis=0)
    face = tot[0:2].sum()
    s12 = tot[2:4].sum()
    se = tot[4:8].sum()
    bg = tot[8:10].sum()
    scale = 1.0 + 1.0 / face
    diff_box = scale * se / (face * 4.0)
    diff_c = scale * (-s12) / face
    diff_bg = ALPHA * (-bg) / (B * N)
    return np.asarray(diff_box + diff_c + diff_bg, dtype=np.float32)


def kernel(x, y, **run_kwargs):
    nc = _get_nc()
    res = run_bass_kernel_spmd(nc, _in_maps(x, y), core_ids=list(range(M)),
                               **run_kwargs)
    out = _combine([res.results[i]["o"] for i in range(M)])
    if run_kwargs:
        return out, res
    return out
